# revision 58
# baseline (speedup 1.0000x reference)
"""Trainium2 Bass kernel for nn_BoundaryPredictor1 (8 NeuronCores).

Pipeline:
  Kernel A (8 cores, data-parallel over tokens): per-token MLP gate
      logits = relu(x @ W1 + b1) @ W2   computed in bf16 (exact decisions are
      restored on host by recomputing the few near-threshold tokens in fp64).
  Host: threshold + forced-boundary logic, segment ids, scalars (loss etc).
  Kernel B (8 cores, sample x D-half parallel): segment mean-pool via
      one-hot matmul.  hidden is fed as bf16 hi + bf16 lo (exact one-hot
      weights), so the pooled means are fp32-accurate.
"""

import math

import ml_dtypes
import numpy as np

import concourse.bacc as bacc
import concourse.mybir as mybir
import concourse.tile as tile


class _Runner:
    """Compile a Bass module once into a persistent jitted 8-core executable."""

    def __init__(self, nc, n_cores):
        import jax
        import jax.numpy as jnp
        from jax.experimental.shard_map import shard_map
        from jax.sharding import Mesh, PartitionSpec
        import concourse.mybir as mybir
        from concourse.bass2jax import (
            _bass_exec_p,
            install_neuronx_cc_hook,
            partition_id_tensor,
        )

        install_neuronx_cc_hook()
        self.nc = nc
        self.n_cores = n_cores
        partition_name = nc.partition_id_tensor.name if nc.partition_id_tensor else None
        in_names, out_names, out_avals = [], [], []
        self.out_shapes = []
        for alloc in nc.m.functions[0].allocations:
            if not isinstance(alloc, mybir.MemoryLocationSet):
                continue
            name = alloc.memorylocations[0].name
            if alloc.kind == "ExternalInput":
                if name != partition_name:
                    in_names.append(name)
            elif alloc.kind == "ExternalOutput":
                out_names.append(name)
                shape = tuple(alloc.tensor_shape)
                dtype = mybir.dt.np(alloc.dtype)
                out_avals.append(jax.core.ShapedArray(shape, dtype))
                self.out_shapes.append((shape, dtype))
        n_params = len(in_names)
        self.in_names = list(in_names)
        self.out_names = out_names
        all_names = in_names + out_names
        if partition_name is not None:
            all_names.append(partition_name)

        def _body(*args):
            operands = list(args)
            if partition_name is not None:
                operands.append(partition_id_tensor())
            outs = _bass_exec_p.bind(
                *operands,
                out_avals=tuple(out_avals),
                in_names=tuple(all_names),
                out_names=tuple(out_names),
                lowering_input_output_aliases=(),
                sim_require_finite=True,
                sim_require_nnan=True,
                nc=nc,
            )
            return tuple(outs)

        devices = jax.devices()[:n_cores]
        mesh = Mesh(np.asarray(devices), ("core",))
        n_outs = len(out_names)
        in_specs = (PartitionSpec("core"),) * (n_params + n_outs)
        out_specs = (PartitionSpec("core"),) * n_outs
        donate = tuple(range(n_params, n_params + n_outs))
        self._fn = jax.jit(
            shard_map(_body, mesh=mesh, in_specs=in_specs, out_specs=out_specs,
                      check_rep=False),
            donate_argnums=donate,
            keep_unused=True,
        )
        self._jnp = jnp

    def _zeros(self):
        return [np.zeros((self.n_cores * s[0], *s[1:]), d) for s, d in self.out_shapes]

    def run_concat(self, concat_in):
        """concat_in: list matching in_names, each (n_cores*dim0, ...). Returns jax arrays."""
        self._bench_inputs = concat_in
        return self._fn(*concat_in, *self._zeros())

    def __call__(self, in_maps):
        concat_in = [
            np.concatenate([np.asarray(m[name]) for m in in_maps], axis=0)
            for name in self.in_names
        ]
        out_arrs = self.run_concat(concat_in)
        return [
            {
                name: np.asarray(out_arrs[i]).reshape(
                    self.n_cores, *self.out_shapes[i][0])[c]
                for i, name in enumerate(self.out_names)
            }
            for c in range(self.n_cores)
        ]

BS, S, D, H = 4, 2048, 1024, 4096
N_CORES = 8
TOK = BS * S // N_CORES  # tokens per core in kernel A (1024)
DH = D // 2              # D-half per core in kernel B (512)
P = 128
TEMP, THRESH, PRIOR, EPS = 1.0, 0.5, 0.2, 1e-9
FIXUP_DELTA = 0.05       # |z| below this gets exact host recompute (~200 tokens)

BF16 = ml_dtypes.bfloat16

_cache = {}


def _build_mlp_nc(TOK=TOK):
    """Kernel A: per core, logits[t] = relu(x[t] @ W1 + b1) @ W2 for TOK tokens.

    x arrives pre-transposed ([D, TOK]) so SBUF loads are plain row DMAs; W1
    streams on the ACT HWDGE ring so it does not queue behind the x loads.
    """
    nc = bacc.Bacc("TRN2", target_bir_lowering=False)
    x_in = nc.declare_dram_parameter("xbfT", [D, TOK], mybir.dt.bfloat16, isOutput=False)
    w1_in = nc.declare_dram_parameter("w1", [D, H], mybir.dt.bfloat16, isOutput=False)
    w2_in = nc.declare_dram_parameter("w2", [P, H // P], mybir.dt.bfloat16, isOutput=False)
    b1_in = nc.declare_dram_parameter("b1", [P, H // P], mybir.dt.float32, isOutput=False)
    z_out = nc.declare_dram_parameter("logits", [1, TOK], mybir.dt.float32, isOutput=True)

    KT = D // P       # 8 k-tiles
    MT = H // P       # 32 m-tiles (H)
    NPAN = 8          # W1 H-panels of 512
    MPP = MT // NPAN  # m-tiles per panel (4)
    # output free-dim chunks (<=512 fp32 per PSUM bank)
    NSPLITS = [(i, min(512, TOK - i)) for i in range(0, TOK, 512)]

    with tile.TileContext(nc) as tc:
        with (
            tc.tile_pool(name="big", bufs=1) as big,
            tc.tile_pool(name="w1p", bufs=4) as w1pool,
            tc.tile_pool(name="ps", bufs=2, space="PSUM") as psum,
            tc.tile_pool(name="ps2", bufs=1, space="PSUM") as psum2,
            tc.tile_pool(name="small", bufs=1) as small,
        ):
            # PE clock-ramp warmup: dummy matmuls on a memset tile while the
            # first DMAs are in flight
            warm = small.tile([P, 64], mybir.dt.bfloat16)
            nc.vector.memset(warm[:], 0.0)
            wps = psum2.tile([P, 64], mybir.dt.float32, tag="warm")
            for _ in range(40):
                nc.tensor.matmul(wps[0:64, :], warm[:], warm[:], start=True, stop=True)

            # first W1 quarter-panel goes out first on the ACT ring (it gates
            # the first matmul); x^T chunks stream on the SP ring in parallel.
            # W1 moves in 2-k-tile quarter panels so x^T chunks interleave into
            # the shared DMA engines instead of queueing behind 0.5MB blocks.
            KQ = 2
            NQ = KT // KQ

            def w1q_dma(tile_, pan, q):
                nc.scalar.dma_start(
                    out=tile_[:],
                    in_=w1_in[q * KQ * P:(q + 1) * KQ * P,
                              pan * MPP * P:(pan + 1) * MPP * P].rearrange(
                        "(k p) m -> p k m", p=P))

            w1_first = w1pool.tile([P, KQ, MPP * P], mybir.dt.bfloat16, tag="w1q0")
            w1q_dma(w1_first, 0, 0)

            # x^T in SBUF as separate 2-k-tile chunks so the first matmuls only
            # wait on the first chunk's DMA
            xt_chunks = []
            for k0 in range(0, KT, 2):
                xc = big.tile([P, 2, TOK], mybir.dt.bfloat16, tag=f"xt{k0}")
                nc.sync.dma_start(
                    out=xc[:],
                    in_=x_in[k0 * P:(k0 + 2) * P, :].rearrange("(k p) t -> p k t", p=P))
                xt_chunks.append(xc)

            def xt(k):
                return xt_chunks[k // 2][:, k % 2, :]
            w2s = small.tile([P, MT], mybir.dt.bfloat16)
            nc.sync.dma_start(out=w2s[:], in_=w2_in[:])
            b1s = small.tile([P, MT], mybir.dt.float32)
            nc.sync.dma_start(out=b1s[:], in_=b1_in[:])

            # h^T in SBUF: [128 H-part, 32 m, 1024 tok] bf16
            h_all = big.tile([P, MT, TOK], mybir.dt.bfloat16)

            for pan in range(NPAN):
                quarters = []
                for q in range(NQ):
                    if pan == 0 and q == 0:
                        t = w1_first
                    else:
                        t = w1pool.tile([P, KQ, MPP * P], mybir.dt.bfloat16,
                                        tag=f"w1q{q}")
                        w1q_dma(t, pan, q)
                    quarters.append(t)
                for mi in range(MPP):
                    m = pan * MPP + mi
                    ps = psum.tile([P, TOK], mybir.dt.float32, tag="ps")
                    for k in range(KT):
                        lhsT = quarters[k // KQ][:, k % KQ, mi * P:(mi + 1) * P]
                        xk = xt(k)
                        for n0, nl in NSPLITS:
                            nc.tensor.matmul(ps[:, n0:n0 + nl], lhsT, xk[:, n0:n0 + nl],
                                             start=(k == 0), stop=(k == KT - 1))
                    # h = relu(r + b1) -> bf16
                    nc.scalar.activation(out=h_all[:, m, :], in_=ps[:],
                                         func=mybir.ActivationFunctionType.Relu,
                                         bias=b1s[:, m:m + 1], scale=1.0)

            # stage 2: logits = sum_m W2[m]^T @ h[m]
            zps = psum2.tile([P, TOK], mybir.dt.float32, tag="zps")
            for m in range(MT):
                for n0, nl in NSPLITS:
                    nc.tensor.matmul(zps[0:1, n0:n0 + nl], w2s[:, m:m + 1],
                                     h_all[:, m, n0:n0 + nl],
                                     start=(m == 0), stop=(m == MT - 1))
            zsb = small.tile([1, TOK], mybir.dt.float32)
            nc.vector.tensor_copy(out=zsb[:], in_=zps[0:1, :])
            nc.sync.dma_start(out=z_out[:], in_=zsb[:])
    nc.compile()
    return nc


def _build_pool_nc(pattern):
    """Kernel B: per core, pooled[s, :] = mean of hidden-half over tokens with seg id s.

    pattern: tuple of (m, (k0, k1, ...)) — for each computed segment tile m, the
    token tiles whose segment-id range intersects it (a diagonal band since
    segment ids are monotone in token position).  Same program runs on all 8
    cores; the pattern is the union over the 4 samples.
    """
    nc = bacc.Bacc("TRN2", target_bir_lowering=False)
    mt = max(m for m, _ in pattern) + 1
    SEG = mt * P
    xh_in = nc.declare_dram_parameter("xh", [S, DH], mybir.dt.bfloat16, isOutput=False)
    xl_in = nc.declare_dram_parameter("xl", [S, DH], mybir.dt.bfloat16, isOutput=False)
    hh1_in = nc.declare_dram_parameter("hh1", [P, S // P], mybir.dt.float32, isOutput=False)
    invc_in = nc.declare_dram_parameter("invc", [P, mt], mybir.dt.float32, isOutput=False)
    iota_in = nc.declare_dram_parameter("iota", [P, SEG], mybir.dt.float32, isOutput=False)
    out = nc.declare_dram_parameter("pooled", [SEG, DH], mybir.dt.float32, isOutput=True)

    KT = S // P  # 16 token tiles
    # per token-tile k: contiguous run of segment tiles it feeds
    k_runs = {}
    for m, ks in pattern:
        for k in ks:
            lo, hi = k_runs.get(k, (m, m))
            k_runs[k] = (min(lo, m), max(hi, m))

    with tile.TileContext(nc) as tc:
        with (
            tc.tile_pool(name="big", bufs=1) as big,
            tc.tile_pool(name="ps", bufs=6, space="PSUM") as psum,
            tc.tile_pool(name="pw", bufs=1, space="PSUM") as psumw,
            tc.tile_pool(name="outp", bufs=4) as outp,
            tc.tile_pool(name="small", bufs=1) as small,
        ):
            warm = small.tile([P, 64], mybir.dt.bfloat16)
            nc.vector.memset(warm[:], 0.0)
            wps = psumw.tile([P, 64], mybir.dt.float32, tag="warm")
            for _ in range(40):
                nc.tensor.matmul(wps[0:64, :], warm[:], warm[:], start=True, stop=True)

            iota_rep = small.tile([P, SEG], mybir.dt.float32)
            nc.scalar.dma_start(out=iota_rep[:], in_=iota_in[:])
            hh1s = small.tile([P, KT], mybir.dt.float32)
            nc.scalar.dma_start(out=hh1s[:], in_=hh1_in[:])

            xh_chunks, xl_chunks = [], []
            for k0 in range(0, KT, 2):
                xhc = big.tile([P, 2, DH], mybir.dt.bfloat16, tag=f"xh{k0}")
                nc.sync.dma_start(
                    out=xhc[:],
                    in_=xh_in[k0 * P:(k0 + 2) * P, :].rearrange("(k p) d -> p k d", p=P))
                xlc = big.tile([P, 2, DH], mybir.dt.bfloat16, tag=f"xl{k0}")
                nc.sync.dma_start(
                    out=xlc[:],
                    in_=xl_in[k0 * P:(k0 + 2) * P, :].rearrange("(k p) d -> p k d", p=P))
                xh_chunks.append(xhc)
                xl_chunks.append(xlc)

            def xh(k):
                return xh_chunks[k // 2][:, k % 2, :]

            def xl(k):
                return xl_chunks[k // 2][:, k % 2, :]

            invcs = small.tile([P, mt], mybir.dt.float32)
            nc.scalar.dma_start(out=invcs[:], in_=invc_in[:])

            # one-hot band tiles: per k, one DVE op covering its segment-tile run
            maxrun = max(hi - lo + 1 for lo, hi in k_runs.values())
            ohk = big.tile([P, KT, maxrun * P], mybir.dt.bfloat16)
            oh = {}
            for k, (mlo, mhi) in sorted(k_runs.items()):
                nrun = mhi - mlo + 1
                nc.vector.tensor_scalar(out=ohk[:, k, :nrun * P],
                                        in0=iota_rep[:, mlo * P:(mhi + 1) * P],
                                        scalar1=hh1s[:, k:k + 1], scalar2=None,
                                        op0=mybir.AluOpType.is_equal)
                for m in range(mlo, mhi + 1):
                    oh[(m, k)] = ohk[:, k, (m - mlo) * P:(m - mlo + 1) * P]

            for m, ks in pattern:
                ps = psum.tile([P, DH], mybir.dt.float32, tag="ps")
                for j, k in enumerate(ks):
                    lhsT = oh[(m, k)][:]
                    nc.tensor.matmul(ps[:], lhsT, xh(k),
                                     start=(j == 0), stop=False)
                    nc.tensor.matmul(ps[:], lhsT, xl(k),
                                     start=False, stop=(j == len(ks) - 1))
                po = outp.tile([P, DH], mybir.dt.float32, tag="po")
                nc.scalar.activation(out=po[:], in_=ps[:],
                                     func=mybir.ActivationFunctionType.Copy,
                                     scale=invcs[:, m:m + 1])
                nc.sync.dma_start(out=out[m * P:(m + 1) * P, :], in_=po[:])
    nc.compile()
    return nc


TOK_FAST = 744            # tokens per core on the packed fast path
MAX_LEFTOVER = 256        # host-side exact-MLP budget for unpacked valid tokens


def kernel(hidden, attention_mask, noise_u, W1, b1, W2, b2):
    hidden = np.asarray(hidden, np.float32)
    attention_mask = np.asarray(attention_mask, np.float32)
    noise_u = np.asarray(noise_u, np.float32)
    W1 = np.asarray(W1, np.float32)
    b1 = np.asarray(b1, np.float32)
    W2 = np.asarray(W2, np.float32)
    b2 = np.float32(np.asarray(b2))

    # ---- host preprocessing -------------------------------------------------
    xflat = hidden.reshape(BS * S, D)
    xbf = xflat.astype(BF16)
    xlo = (xflat - xbf.astype(np.float32)).astype(BF16)
    w1bf = W1.astype(BF16)
    w2t = W2.astype(BF16).reshape(H // P, P).T.copy()       # [128, 32]
    b1t = b1.reshape(H // P, P).T.copy()                    # [128, 32] f32

    # ---- kernel A: logits ---------------------------------------------------
    valid_flat = (attention_mask.reshape(-1) > 0.0)
    vidx = np.nonzero(valid_flat)[0]
    V = vidx.size
    fast = V - N_CORES * TOK_FAST <= MAX_LEFTOVER
    tok_a = TOK_FAST if fast else TOK
    key_a = ("mlp", tok_a)
    if key_a not in _cache:
        _cache[key_a] = _Runner(_build_mlp_nc(tok_a), N_CORES)
    run_a = _cache[key_a]

    nbatch = N_CORES * tok_a
    if fast:
        dev_idx = vidx[:nbatch]
        if dev_idx.size < nbatch:  # pad with token 0
            dev_idx = np.concatenate(
                [dev_idx, np.zeros(nbatch - dev_idx.size, np.int64)])
        host_idx = vidx[nbatch:]
        xa = xbf[dev_idx]
    else:
        dev_idx = np.arange(nbatch)
        host_idx = np.zeros(0, np.int64)
        xa = xbf
    in_maps = [
        {
            "xbfT": np.ascontiguousarray(xa[c * tok_a:(c + 1) * tok_a].T),
            "w1": w1bf,
            "w2": w2t,
            "b1": b1t,
        }
        for c in range(N_CORES)
    ]
    res_a = run_a(in_maps)
    zdev = np.concatenate([res_a[c]["logits"][0] for c in range(N_CORES)])

    n_real = min(nbatch, V) if fast else nbatch
    logits = np.zeros(BS * S, np.float64)
    logits[dev_idx[:n_real]] = zdev.astype(np.float64)[:n_real]
    W1_64 = None
    if host_idx.size:
        W1_64 = W1.astype(np.float64)
        xr = hidden.reshape(-1, D)[host_idx].astype(np.float64)
        hr = np.maximum(xr @ W1_64 + b1.astype(np.float64), 0.0)
        logits[host_idx] = hr @ W2.astype(np.float64)
    logits = logits.reshape(BS, S)

    # ---- host: exact decisions ---------------------------------------------
    noise64 = noise_u.astype(np.float64)
    logistic = np.log(noise64) - np.log1p(-noise64)
    z = logits + float(b2) + logistic
    # recompute near-threshold tokens exactly (fp64).  The device bf16 logit
    # error scales with the logit magnitude; 0.03*std(z) keeps a ~5x margin
    # over the measured max error for unit-scale inputs.
    zvalid = z[attention_mask > 0.0]
    delta = max(FIXUP_DELTA, 0.03 * float(np.std(zvalid)) if zvalid.size else 0.0)
    risky = (np.abs(z) < delta) & (attention_mask > 0.0)
    if risky.any():
        rb, rs = np.nonzero(risky)
        if W1_64 is None:
            W1_64 = W1.astype(np.float64)
        xr = hidden[rb, rs].astype(np.float64)              # [n, D]
        hr = np.maximum(xr @ W1_64 + b1.astype(np.float64), 0.0)
        zr = hr @ W2.astype(np.float64) + float(b2) + logistic[rb, rs]
        z[rb, rs] = zr

    hard = ((z > 0.0) & (attention_mask > 0.0)).astype(np.int64)
    # forced boundary on last real token of each row (only when row has padding)
    lens = (attention_mask > 0.0).sum(1).astype(np.int64)
    for bi in range(BS):
        if 0 < lens[bi] < S:
            hard[bi, lens[bi] - 1] = 1
    hh1 = np.cumsum(hard, axis=1) - hard                    # segment id per token
    nseg = hh1[:, -1] + 1                                   # segments incl. trailing pad segment
    counts = np.stack([np.bincount(hh1[bi], minlength=S) for bi in range(BS)])

    num_b = np.float32(hard.sum())
    total = np.float32(attention_mask.sum())
    lg = math.lgamma
    log_prob = (lg(float(total) + 1.0) - lg(float(num_b) + 1.0)
                - lg(float(total) - float(num_b) + 1.0)
                + float(num_b) * math.log(PRIOR)
                + (float(total) - float(num_b)) * math.log(1.0 - PRIOR))
    loss = np.float32(-log_prob / float(total))
    counts_rows = hard.sum(1).astype(np.float32)
    short_mask = (np.arange(S, dtype=np.float32)[None, :] < counts_rows[:, None]).astype(np.float32)

    # ---- kernel B: segment mean-pool ---------------------------------------
    mt = (int(nseg.max()) + P - 1) // P
    SEG = mt * P
    # band pattern: union over samples of (segment-tile m, token-tile k) overlaps
    KT = S // P
    pairs = set()
    for bi in range(BS):
        lo = hh1[bi].reshape(KT, P).min(1) // P
        hi = hh1[bi].reshape(KT, P).max(1) // P
        for k in range(KT):
            for m in range(int(lo[k]), int(hi[k]) + 1):
                pairs.add((m, k))
    pattern = tuple(
        (m, tuple(sorted(k for mm, k in pairs if mm == m)))
        for m in sorted({mm for mm, _ in pairs})
    )
    key = ("pool", pattern)
    if key not in _cache:
        _cache[key] = _Runner(_build_pool_nc(pattern), N_CORES)
    run_b = _cache[key]

    invc = (np.float32(1.0) /
            (counts[:, :SEG].astype(np.float32) + np.float32(EPS)))  # [BS, SEG]
    hh1f = hh1.astype(np.float32)
    xbf3 = xbf.reshape(BS, S, D)
    xlo3 = xlo.reshape(BS, S, D)
    iota = np.ascontiguousarray(
        np.broadcast_to(np.arange(SEG, dtype=np.float32), (P, SEG)))

    in_maps_b = []
    for c in range(N_CORES):
        bi, half = c // 2, c % 2
        d0 = half * DH
        in_maps_b.append({
            "xh": np.ascontiguousarray(xbf3[bi, :, d0:d0 + DH]),
            "xl": np.ascontiguousarray(xlo3[bi, :, d0:d0 + DH]),
            "hh1": np.ascontiguousarray(hh1f[bi].reshape(KT, P).T),
            "invc": np.ascontiguousarray(invc[bi].reshape(mt, P).T),
            "iota": iota,
        })
    res_b = run_b(in_maps_b)

    pooled = np.zeros((BS, S, D), np.float32)
    for c in range(N_CORES):
        bi, half = c // 2, c % 2
        d0 = half * DH
        pooled[bi, :SEG, d0:d0 + DH] = res_b[c]["pooled"]

    return pooled, loss, num_b, total, short_mask


# revision 60
# speedup vs baseline: 1.0546x; 1.0546x over previous
"""Trainium2 Bass kernel for nn_BoundaryPredictor1 (8 NeuronCores).

Pipeline:
  Kernel A (8 cores, data-parallel over tokens): per-token MLP gate
      logits = relu(x @ W1 + b1) @ W2   computed in bf16 (exact decisions are
      restored on host by recomputing the few near-threshold tokens in fp64).
  Host: threshold + forced-boundary logic, segment ids, scalars (loss etc).
  Kernel B (8 cores, sample x D-half parallel): segment mean-pool via
      one-hot matmul.  hidden is fed as bf16 hi + bf16 lo (exact one-hot
      weights), so the pooled means are fp32-accurate.
"""

import math

import ml_dtypes
import numpy as np

import concourse.bacc as bacc
import concourse.mybir as mybir
import concourse.tile as tile


class _Runner:
    """Compile a Bass module once into a persistent jitted 8-core executable."""

    def __init__(self, nc, n_cores):
        import jax
        import jax.numpy as jnp
        from jax.experimental.shard_map import shard_map
        from jax.sharding import Mesh, PartitionSpec
        import concourse.mybir as mybir
        from concourse.bass2jax import (
            _bass_exec_p,
            install_neuronx_cc_hook,
            partition_id_tensor,
        )

        install_neuronx_cc_hook()
        self.nc = nc
        self.n_cores = n_cores
        partition_name = nc.partition_id_tensor.name if nc.partition_id_tensor else None
        in_names, out_names, out_avals = [], [], []
        self.out_shapes = []
        for alloc in nc.m.functions[0].allocations:
            if not isinstance(alloc, mybir.MemoryLocationSet):
                continue
            name = alloc.memorylocations[0].name
            if alloc.kind == "ExternalInput":
                if name != partition_name:
                    in_names.append(name)
            elif alloc.kind == "ExternalOutput":
                out_names.append(name)
                shape = tuple(alloc.tensor_shape)
                dtype = mybir.dt.np(alloc.dtype)
                out_avals.append(jax.core.ShapedArray(shape, dtype))
                self.out_shapes.append((shape, dtype))
        n_params = len(in_names)
        self.in_names = list(in_names)
        self.out_names = out_names
        all_names = in_names + out_names
        if partition_name is not None:
            all_names.append(partition_name)

        def _body(*args):
            operands = list(args)
            if partition_name is not None:
                operands.append(partition_id_tensor())
            outs = _bass_exec_p.bind(
                *operands,
                out_avals=tuple(out_avals),
                in_names=tuple(all_names),
                out_names=tuple(out_names),
                lowering_input_output_aliases=(),
                sim_require_finite=True,
                sim_require_nnan=True,
                nc=nc,
            )
            return tuple(outs)

        devices = jax.devices()[:n_cores]
        mesh = Mesh(np.asarray(devices), ("core",))
        n_outs = len(out_names)
        in_specs = (PartitionSpec("core"),) * (n_params + n_outs)
        out_specs = (PartitionSpec("core"),) * n_outs
        donate = tuple(range(n_params, n_params + n_outs))
        self._fn = jax.jit(
            shard_map(_body, mesh=mesh, in_specs=in_specs, out_specs=out_specs,
                      check_rep=False),
            donate_argnums=donate,
            keep_unused=True,
        )
        self._jnp = jnp

    def _zeros(self):
        return [np.zeros((self.n_cores * s[0], *s[1:]), d) for s, d in self.out_shapes]

    def run_concat(self, concat_in):
        """concat_in: list matching in_names, each (n_cores*dim0, ...). Returns jax arrays."""
        self._bench_inputs = concat_in
        return self._fn(*concat_in, *self._zeros())

    def __call__(self, in_maps):
        concat_in = [
            np.concatenate([np.asarray(m[name]) for m in in_maps], axis=0)
            for name in self.in_names
        ]
        out_arrs = self.run_concat(concat_in)
        return [
            {
                name: np.asarray(out_arrs[i]).reshape(
                    self.n_cores, *self.out_shapes[i][0])[c]
                for i, name in enumerate(self.out_names)
            }
            for c in range(self.n_cores)
        ]

BS, S, D, H = 4, 2048, 1024, 4096
N_CORES = 8
TOK = BS * S // N_CORES  # tokens per core in kernel A (1024)
DH = D // 2              # D-half per core in kernel B (512)
P = 128
TEMP, THRESH, PRIOR, EPS = 1.0, 0.5, 0.2, 1e-9
FIXUP_DELTA = 0.05       # |z| below this gets exact host recompute (~200 tokens)

BF16 = ml_dtypes.bfloat16

_cache = {}


def _build_mlp_nc(TOK=TOK):
    """Kernel A: per core, logits[t] = relu(x[t] @ W1 + b1) @ W2 for TOK tokens.

    x arrives pre-transposed ([D, TOK]) so SBUF loads are plain row DMAs; W1
    streams on the ACT HWDGE ring so it does not queue behind the x loads.
    """
    nc = bacc.Bacc("TRN2", target_bir_lowering=False)
    x_in = nc.declare_dram_parameter("xbfT", [D, TOK], mybir.dt.bfloat16, isOutput=False)
    w1_in = nc.declare_dram_parameter("w1", [D, H], mybir.dt.bfloat16, isOutput=False)
    w2_in = nc.declare_dram_parameter("w2", [P, H // P], mybir.dt.float32, isOutput=False)
    b1_in = nc.declare_dram_parameter("b1", [P, H // P], mybir.dt.float32, isOutput=False)
    z_out = nc.declare_dram_parameter("logits", [1, TOK], mybir.dt.float32, isOutput=True)

    KT = D // P       # 8 k-tiles
    MT = H // P       # 32 m-tiles (H)
    NPAN = 8          # W1 H-panels of 512
    MPP = MT // NPAN  # m-tiles per panel (4)
    # output free-dim chunks (<=512 fp32 per PSUM bank)
    NSPLITS = [(i, min(512, TOK - i)) for i in range(0, TOK, 512)]

    with tile.TileContext(nc) as tc:
        with (
            tc.tile_pool(name="big", bufs=1) as big,
            tc.tile_pool(name="w1p", bufs=4) as w1pool,
            tc.tile_pool(name="ps", bufs=2, space="PSUM") as psum,
            tc.tile_pool(name="ps2", bufs=1, space="PSUM") as psum2,
            tc.tile_pool(name="small", bufs=1) as small,
        ):
            # PE clock-ramp warmup: dummy matmuls on a memset tile while the
            # first DMAs are in flight
            warm = small.tile([P, 64], mybir.dt.bfloat16)
            nc.vector.memset(warm[:], 0.0)
            wps = psum2.tile([P, 64], mybir.dt.float32, tag="warm")
            for _ in range(40):
                nc.tensor.matmul(wps[0:64, :], warm[:], warm[:], start=True, stop=True)

            # first W1 quarter-panel goes out first on the ACT ring (it gates
            # the first matmul); x^T chunks stream on the SP ring in parallel.
            # W1 moves in 2-k-tile quarter panels so x^T chunks interleave into
            # the shared DMA engines instead of queueing behind 0.5MB blocks.
            KQ = 2
            NQ = KT // KQ

            def w1q_dma(tile_, pan, q):
                nc.scalar.dma_start(
                    out=tile_[:],
                    in_=w1_in[q * KQ * P:(q + 1) * KQ * P,
                              pan * MPP * P:(pan + 1) * MPP * P].rearrange(
                        "(k p) m -> p k m", p=P))

            w1_first = w1pool.tile([P, KQ, MPP * P], mybir.dt.bfloat16, tag="w1q0")
            w1q_dma(w1_first, 0, 0)

            # x^T in SBUF as separate 2-k-tile chunks so the first matmuls only
            # wait on the first chunk's DMA
            xt_chunks = []
            for k0 in range(0, KT, 2):
                xc = big.tile([P, 2, TOK], mybir.dt.bfloat16, tag=f"xt{k0}")
                nc.sync.dma_start(
                    out=xc[:],
                    in_=x_in[k0 * P:(k0 + 2) * P, :].rearrange("(k p) t -> p k t", p=P))
                xt_chunks.append(xc)

            def xt(k):
                return xt_chunks[k // 2][:, k % 2, :]
            w2s = small.tile([P, MT], mybir.dt.float32)
            nc.sync.dma_start(out=w2s[:], in_=w2_in[:])
            b1s = small.tile([P, MT], mybir.dt.float32)
            nc.sync.dma_start(out=b1s[:], in_=b1_in[:])

            # stage-2 runs on the DVE: acc[p,t] += h_m[p,t] * W2[128m+p],
            # then one ones-vector matmul reduces acc over partitions.
            acc = big.tile([P, TOK], mybir.dt.float32)
            ones = small.tile([P, 1], mybir.dt.float32)
            nc.vector.memset(ones[:], 1.0)

            for pan in range(NPAN):
                quarters = []
                for q in range(NQ):
                    if pan == 0 and q == 0:
                        t = w1_first
                    else:
                        t = w1pool.tile([P, KQ, MPP * P], mybir.dt.bfloat16,
                                        tag=f"w1q{q}")
                        w1q_dma(t, pan, q)
                    quarters.append(t)
                for mi in range(MPP):
                    m = pan * MPP + mi
                    ps = psum.tile([P, TOK], mybir.dt.float32, tag="ps")
                    for k in range(KT):
                        lhsT = quarters[k // KQ][:, k % KQ, mi * P:(mi + 1) * P]
                        xk = xt(k)
                        for n0, nl in NSPLITS:
                            nc.tensor.matmul(ps[:, n0:n0 + nl], lhsT, xk[:, n0:n0 + nl],
                                             start=(k == 0), stop=(k == KT - 1))
                    # h = relu(r + b1) -> bf16
                    hm = w1pool.tile([P, TOK], mybir.dt.bfloat16, tag="hm")
                    nc.scalar.activation(out=hm[:], in_=ps[:],
                                         func=mybir.ActivationFunctionType.Relu,
                                         bias=b1s[:, m:m + 1], scale=1.0)
                    if m == 0:
                        nc.vector.tensor_scalar_mul(out=acc[:], in0=hm[:],
                                                    scalar1=w2s[:, 0:1])
                    else:
                        nc.vector.scalar_tensor_tensor(
                            out=acc[:], in0=hm[:], scalar=w2s[:, m:m + 1],
                            in1=acc[:], op0=mybir.AluOpType.mult,
                            op1=mybir.AluOpType.add)

            # stage 2 partition-reduction: logits = ones^T @ acc
            zps = psum2.tile([P, TOK], mybir.dt.float32, tag="zps")
            for n0, nl in NSPLITS:
                nc.tensor.matmul(zps[0:1, n0:n0 + nl], ones[:], acc[:, n0:n0 + nl],
                                 start=True, stop=True)
            zsb = small.tile([1, TOK], mybir.dt.float32)
            nc.vector.tensor_copy(out=zsb[:], in_=zps[0:1, :])
            nc.sync.dma_start(out=z_out[:], in_=zsb[:])
    nc.compile()
    return nc


def _build_pool_nc(pattern):
    """Kernel B: per core, pooled[s, :] = mean of hidden-half over tokens with seg id s.

    pattern: tuple of (m, (k0, k1, ...)) — for each computed segment tile m, the
    token tiles whose segment-id range intersects it (a diagonal band since
    segment ids are monotone in token position).  Same program runs on all 8
    cores; the pattern is the union over the 4 samples.
    """
    nc = bacc.Bacc("TRN2", target_bir_lowering=False)
    mt = max(m for m, _ in pattern) + 1
    SEG = mt * P
    xh_in = nc.declare_dram_parameter("xh", [S, DH], mybir.dt.bfloat16, isOutput=False)
    xl_in = nc.declare_dram_parameter("xl", [S, DH], mybir.dt.bfloat16, isOutput=False)
    hh1_in = nc.declare_dram_parameter("hh1", [P, S // P], mybir.dt.float32, isOutput=False)
    invc_in = nc.declare_dram_parameter("invc", [P, mt], mybir.dt.float32, isOutput=False)
    iota_in = nc.declare_dram_parameter("iota", [P, SEG], mybir.dt.float32, isOutput=False)
    out = nc.declare_dram_parameter("pooled", [SEG, DH], mybir.dt.float32, isOutput=True)

    KT = S // P  # 16 token tiles
    # per token-tile k: contiguous run of segment tiles it feeds
    k_runs = {}
    for m, ks in pattern:
        for k in ks:
            lo, hi = k_runs.get(k, (m, m))
            k_runs[k] = (min(lo, m), max(hi, m))

    with tile.TileContext(nc) as tc:
        with (
            tc.tile_pool(name="big", bufs=1) as big,
            tc.tile_pool(name="ps", bufs=6, space="PSUM") as psum,
            tc.tile_pool(name="pw", bufs=1, space="PSUM") as psumw,
            tc.tile_pool(name="outp", bufs=4) as outp,
            tc.tile_pool(name="small", bufs=1) as small,
        ):
            warm = small.tile([P, 64], mybir.dt.bfloat16)
            nc.vector.memset(warm[:], 0.0)
            wps = psumw.tile([P, 64], mybir.dt.float32, tag="warm")
            for _ in range(40):
                nc.tensor.matmul(wps[0:64, :], warm[:], warm[:], start=True, stop=True)

            iota_rep = small.tile([P, SEG], mybir.dt.float32)
            nc.scalar.dma_start(out=iota_rep[:], in_=iota_in[:])
            hh1s = small.tile([P, KT], mybir.dt.float32)
            nc.scalar.dma_start(out=hh1s[:], in_=hh1_in[:])

            xh_chunks, xl_chunks = [], []
            for k0 in range(0, KT, 2):
                xhc = big.tile([P, 2, DH], mybir.dt.bfloat16, tag=f"xh{k0}")
                nc.sync.dma_start(
                    out=xhc[:],
                    in_=xh_in[k0 * P:(k0 + 2) * P, :].rearrange("(k p) d -> p k d", p=P))
                xlc = big.tile([P, 2, DH], mybir.dt.bfloat16, tag=f"xl{k0}")
                nc.sync.dma_start(
                    out=xlc[:],
                    in_=xl_in[k0 * P:(k0 + 2) * P, :].rearrange("(k p) d -> p k d", p=P))
                xh_chunks.append(xhc)
                xl_chunks.append(xlc)

            def xh(k):
                return xh_chunks[k // 2][:, k % 2, :]

            def xl(k):
                return xl_chunks[k // 2][:, k % 2, :]

            invcs = small.tile([P, mt], mybir.dt.float32)
            nc.scalar.dma_start(out=invcs[:], in_=invc_in[:])

            # one-hot band tiles: per k, one DVE op covering its segment-tile run
            maxrun = max(hi - lo + 1 for lo, hi in k_runs.values())
            ohk = big.tile([P, KT, maxrun * P], mybir.dt.bfloat16)
            oh = {}
            for k, (mlo, mhi) in sorted(k_runs.items()):
                nrun = mhi - mlo + 1
                nc.vector.tensor_scalar(out=ohk[:, k, :nrun * P],
                                        in0=iota_rep[:, mlo * P:(mhi + 1) * P],
                                        scalar1=hh1s[:, k:k + 1], scalar2=None,
                                        op0=mybir.AluOpType.is_equal)
                for m in range(mlo, mhi + 1):
                    oh[(m, k)] = ohk[:, k, (m - mlo) * P:(m - mlo + 1) * P]

            for m, ks in pattern:
                ps = psum.tile([P, DH], mybir.dt.float32, tag="ps")
                for j, k in enumerate(ks):
                    lhsT = oh[(m, k)][:]
                    nc.tensor.matmul(ps[:], lhsT, xh(k),
                                     start=(j == 0), stop=False)
                    nc.tensor.matmul(ps[:], lhsT, xl(k),
                                     start=False, stop=(j == len(ks) - 1))
                po = outp.tile([P, DH], mybir.dt.float32, tag="po")
                nc.scalar.activation(out=po[:], in_=ps[:],
                                     func=mybir.ActivationFunctionType.Copy,
                                     scale=invcs[:, m:m + 1])
                nc.sync.dma_start(out=out[m * P:(m + 1) * P, :], in_=po[:])
    nc.compile()
    return nc


TOK_FAST = 744            # tokens per core on the packed fast path
MAX_LEFTOVER = 256        # host-side exact-MLP budget for unpacked valid tokens


def kernel(hidden, attention_mask, noise_u, W1, b1, W2, b2):
    hidden = np.asarray(hidden, np.float32)
    attention_mask = np.asarray(attention_mask, np.float32)
    noise_u = np.asarray(noise_u, np.float32)
    W1 = np.asarray(W1, np.float32)
    b1 = np.asarray(b1, np.float32)
    W2 = np.asarray(W2, np.float32)
    b2 = np.float32(np.asarray(b2))

    # ---- host preprocessing -------------------------------------------------
    xflat = hidden.reshape(BS * S, D)
    xbf = xflat.astype(BF16)
    xlo = (xflat - xbf.astype(np.float32)).astype(BF16)
    w1bf = W1.astype(BF16)
    w2t = W2.reshape(H // P, P).T.copy()                    # [128, 32] f32
    b1t = b1.reshape(H // P, P).T.copy()                    # [128, 32] f32

    # ---- kernel A: logits ---------------------------------------------------
    valid_flat = (attention_mask.reshape(-1) > 0.0)
    vidx = np.nonzero(valid_flat)[0]
    V = vidx.size
    fast = V - N_CORES * TOK_FAST <= MAX_LEFTOVER
    tok_a = TOK_FAST if fast else TOK
    key_a = ("mlp", tok_a)
    if key_a not in _cache:
        _cache[key_a] = _Runner(_build_mlp_nc(tok_a), N_CORES)
    run_a = _cache[key_a]

    nbatch = N_CORES * tok_a
    if fast:
        dev_idx = vidx[:nbatch]
        if dev_idx.size < nbatch:  # pad with token 0
            dev_idx = np.concatenate(
                [dev_idx, np.zeros(nbatch - dev_idx.size, np.int64)])
        host_idx = vidx[nbatch:]
        xa = xbf[dev_idx]
    else:
        dev_idx = np.arange(nbatch)
        host_idx = np.zeros(0, np.int64)
        xa = xbf
    in_maps = [
        {
            "xbfT": np.ascontiguousarray(xa[c * tok_a:(c + 1) * tok_a].T),
            "w1": w1bf,
            "w2": w2t,
            "b1": b1t,
        }
        for c in range(N_CORES)
    ]
    res_a = run_a(in_maps)
    zdev = np.concatenate([res_a[c]["logits"][0] for c in range(N_CORES)])

    n_real = min(nbatch, V) if fast else nbatch
    logits = np.zeros(BS * S, np.float64)
    logits[dev_idx[:n_real]] = zdev.astype(np.float64)[:n_real]
    W1_64 = None
    if host_idx.size:
        W1_64 = W1.astype(np.float64)
        xr = hidden.reshape(-1, D)[host_idx].astype(np.float64)
        hr = np.maximum(xr @ W1_64 + b1.astype(np.float64), 0.0)
        logits[host_idx] = hr @ W2.astype(np.float64)
    logits = logits.reshape(BS, S)

    # ---- host: exact decisions ---------------------------------------------
    noise64 = noise_u.astype(np.float64)
    logistic = np.log(noise64) - np.log1p(-noise64)
    z = logits + float(b2) + logistic
    # recompute near-threshold tokens exactly (fp64).  The device bf16 logit
    # error scales with the logit magnitude; 0.03*std(z) keeps a ~5x margin
    # over the measured max error for unit-scale inputs.
    zvalid = z[attention_mask > 0.0]
    delta = max(FIXUP_DELTA, 0.03 * float(np.std(zvalid)) if zvalid.size else 0.0)
    risky = (np.abs(z) < delta) & (attention_mask > 0.0)
    if risky.any():
        rb, rs = np.nonzero(risky)
        if W1_64 is None:
            W1_64 = W1.astype(np.float64)
        xr = hidden[rb, rs].astype(np.float64)              # [n, D]
        hr = np.maximum(xr @ W1_64 + b1.astype(np.float64), 0.0)
        zr = hr @ W2.astype(np.float64) + float(b2) + logistic[rb, rs]
        z[rb, rs] = zr

    hard = ((z > 0.0) & (attention_mask > 0.0)).astype(np.int64)
    # forced boundary on last real token of each row (only when row has padding)
    lens = (attention_mask > 0.0).sum(1).astype(np.int64)
    for bi in range(BS):
        if 0 < lens[bi] < S:
            hard[bi, lens[bi] - 1] = 1
    hh1 = np.cumsum(hard, axis=1) - hard                    # segment id per token
    nseg = hh1[:, -1] + 1                                   # segments incl. trailing pad segment
    counts = np.stack([np.bincount(hh1[bi], minlength=S) for bi in range(BS)])

    num_b = np.float32(hard.sum())
    total = np.float32(attention_mask.sum())
    lg = math.lgamma
    log_prob = (lg(float(total) + 1.0) - lg(float(num_b) + 1.0)
                - lg(float(total) - float(num_b) + 1.0)
                + float(num_b) * math.log(PRIOR)
                + (float(total) - float(num_b)) * math.log(1.0 - PRIOR))
    loss = np.float32(-log_prob / float(total))
    counts_rows = hard.sum(1).astype(np.float32)
    short_mask = (np.arange(S, dtype=np.float32)[None, :] < counts_rows[:, None]).astype(np.float32)

    # ---- kernel B: segment mean-pool ---------------------------------------
    mt = (int(nseg.max()) + P - 1) // P
    SEG = mt * P
    # band pattern: union over samples of (segment-tile m, token-tile k) overlaps
    KT = S // P
    pairs = set()
    for bi in range(BS):
        lo = hh1[bi].reshape(KT, P).min(1) // P
        hi = hh1[bi].reshape(KT, P).max(1) // P
        for k in range(KT):
            for m in range(int(lo[k]), int(hi[k]) + 1):
                pairs.add((m, k))
    pattern = tuple(
        (m, tuple(sorted(k for mm, k in pairs if mm == m)))
        for m in sorted({mm for mm, _ in pairs})
    )
    key = ("pool", pattern)
    if key not in _cache:
        _cache[key] = _Runner(_build_pool_nc(pattern), N_CORES)
    run_b = _cache[key]

    invc = (np.float32(1.0) /
            (counts[:, :SEG].astype(np.float32) + np.float32(EPS)))  # [BS, SEG]
    hh1f = hh1.astype(np.float32)
    xbf3 = xbf.reshape(BS, S, D)
    xlo3 = xlo.reshape(BS, S, D)
    iota = np.ascontiguousarray(
        np.broadcast_to(np.arange(SEG, dtype=np.float32), (P, SEG)))

    in_maps_b = []
    for c in range(N_CORES):
        bi, half = c // 2, c % 2
        d0 = half * DH
        in_maps_b.append({
            "xh": np.ascontiguousarray(xbf3[bi, :, d0:d0 + DH]),
            "xl": np.ascontiguousarray(xlo3[bi, :, d0:d0 + DH]),
            "hh1": np.ascontiguousarray(hh1f[bi].reshape(KT, P).T),
            "invc": np.ascontiguousarray(invc[bi].reshape(mt, P).T),
            "iota": iota,
        })
    res_b = run_b(in_maps_b)

    pooled = np.zeros((BS, S, D), np.float32)
    for c in range(N_CORES):
        bi, half = c // 2, c % 2
        d0 = half * DH
        pooled[bi, :SEG, d0:d0 + DH] = res_b[c]["pooled"]

    return pooled, loss, num_b, total, short_mask


# revision 62
# speedup vs baseline: 1.0648x; 1.0097x over previous
"""Trainium2 Bass kernel for nn_BoundaryPredictor1 (8 NeuronCores).

Pipeline:
  Kernel A (8 cores, data-parallel over tokens): per-token MLP gate
      logits = relu(x @ W1 + b1) @ W2   computed in bf16 (exact decisions are
      restored on host by recomputing the few near-threshold tokens in fp64).
  Host: threshold + forced-boundary logic, segment ids, scalars (loss etc).
  Kernel B (8 cores, sample x D-half parallel): segment mean-pool via
      one-hot matmul.  hidden is fed as bf16 hi + bf16 lo (exact one-hot
      weights), so the pooled means are fp32-accurate.
"""

import math

import ml_dtypes
import numpy as np

import concourse.bacc as bacc
import concourse.mybir as mybir
import concourse.tile as tile


class _Runner:
    """Compile a Bass module once into a persistent jitted 8-core executable."""

    def __init__(self, nc, n_cores):
        import jax
        import jax.numpy as jnp
        from jax.experimental.shard_map import shard_map
        from jax.sharding import Mesh, PartitionSpec
        import concourse.mybir as mybir
        from concourse.bass2jax import (
            _bass_exec_p,
            install_neuronx_cc_hook,
            partition_id_tensor,
        )

        install_neuronx_cc_hook()
        self.nc = nc
        self.n_cores = n_cores
        partition_name = nc.partition_id_tensor.name if nc.partition_id_tensor else None
        in_names, out_names, out_avals = [], [], []
        self.out_shapes = []
        for alloc in nc.m.functions[0].allocations:
            if not isinstance(alloc, mybir.MemoryLocationSet):
                continue
            name = alloc.memorylocations[0].name
            if alloc.kind == "ExternalInput":
                if name != partition_name:
                    in_names.append(name)
            elif alloc.kind == "ExternalOutput":
                out_names.append(name)
                shape = tuple(alloc.tensor_shape)
                dtype = mybir.dt.np(alloc.dtype)
                out_avals.append(jax.core.ShapedArray(shape, dtype))
                self.out_shapes.append((shape, dtype))
        n_params = len(in_names)
        self.in_names = list(in_names)
        self.out_names = out_names
        all_names = in_names + out_names
        if partition_name is not None:
            all_names.append(partition_name)

        def _body(*args):
            operands = list(args)
            if partition_name is not None:
                operands.append(partition_id_tensor())
            outs = _bass_exec_p.bind(
                *operands,
                out_avals=tuple(out_avals),
                in_names=tuple(all_names),
                out_names=tuple(out_names),
                lowering_input_output_aliases=(),
                sim_require_finite=True,
                sim_require_nnan=True,
                nc=nc,
            )
            return tuple(outs)

        devices = jax.devices()[:n_cores]
        mesh = Mesh(np.asarray(devices), ("core",))
        n_outs = len(out_names)
        in_specs = (PartitionSpec("core"),) * (n_params + n_outs)
        out_specs = (PartitionSpec("core"),) * n_outs
        donate = tuple(range(n_params, n_params + n_outs))
        self._fn = jax.jit(
            shard_map(_body, mesh=mesh, in_specs=in_specs, out_specs=out_specs,
                      check_rep=False),
            donate_argnums=donate,
            keep_unused=True,
        )
        self._jnp = jnp

    def _zeros(self):
        return [np.zeros((self.n_cores * s[0], *s[1:]), d) for s, d in self.out_shapes]

    def run_concat(self, concat_in):
        """concat_in: list matching in_names, each (n_cores*dim0, ...). Returns jax arrays."""
        self._bench_inputs = concat_in
        return self._fn(*concat_in, *self._zeros())

    def __call__(self, in_maps):
        concat_in = [
            np.concatenate([np.asarray(m[name]) for m in in_maps], axis=0)
            for name in self.in_names
        ]
        out_arrs = self.run_concat(concat_in)
        return [
            {
                name: np.asarray(out_arrs[i]).reshape(
                    self.n_cores, *self.out_shapes[i][0])[c]
                for i, name in enumerate(self.out_names)
            }
            for c in range(self.n_cores)
        ]

BS, S, D, H = 4, 2048, 1024, 4096
N_CORES = 8
TOK = BS * S // N_CORES  # tokens per core in kernel A (1024)
DH = D // 2              # D-half per core in kernel B (512)
P = 128
TEMP, THRESH, PRIOR, EPS = 1.0, 0.5, 0.2, 1e-9
FIXUP_DELTA = 0.05       # |z| below this gets exact host recompute (~200 tokens)

BF16 = ml_dtypes.bfloat16

_cache = {}


def _build_mlp_nc(TOK=TOK):
    """Kernel A: per core, logits[t] = relu(x[t] @ W1 + b1) @ W2 for TOK tokens.

    x arrives pre-transposed ([D, TOK]) so SBUF loads are plain row DMAs; W1
    streams on the ACT HWDGE ring so it does not queue behind the x loads.
    """
    nc = bacc.Bacc("TRN2", target_bir_lowering=False)
    x_in = nc.declare_dram_parameter("xbfT", [D, TOK], mybir.dt.bfloat16, isOutput=False)
    w1_in = nc.declare_dram_parameter("w1", [D, H], mybir.dt.bfloat16, isOutput=False)
    w2_in = nc.declare_dram_parameter("w2", [P, H // P], mybir.dt.float32, isOutput=False)
    b1_in = nc.declare_dram_parameter("b1", [P, H // P], mybir.dt.float32, isOutput=False)
    z_out = nc.declare_dram_parameter("logits", [1, TOK], mybir.dt.float32, isOutput=True)

    KT = D // P       # 8 k-tiles
    MT = H // P       # 32 m-tiles (H)
    NPAN = 8          # W1 H-panels of 512
    MPP = MT // NPAN  # m-tiles per panel (4)
    # output free-dim chunks (<=512 fp32 per PSUM bank)
    NSPLITS = [(i, min(512, TOK - i)) for i in range(0, TOK, 512)]

    with tile.TileContext(nc) as tc:
        with (
            tc.tile_pool(name="big", bufs=1) as big,
            tc.tile_pool(name="w1p", bufs=4) as w1pool,
            tc.tile_pool(name="ps", bufs=2, space="PSUM") as psum,
            tc.tile_pool(name="ps2", bufs=1, space="PSUM") as psum2,
            tc.tile_pool(name="small", bufs=1) as small,
        ):
            # PE clock-ramp warmup: dummy matmuls on a memset tile while the
            # first DMAs are in flight
            warm = small.tile([P, 64], mybir.dt.bfloat16)
            nc.vector.memset(warm[:], 0.0)
            wps = psum2.tile([P, 64], mybir.dt.float32, tag="warm")
            for _ in range(40):
                nc.tensor.matmul(wps[0:64, :], warm[:], warm[:], start=True, stop=True)

            # first W1 quarter-panel goes out first on the ACT ring (it gates
            # the first matmul); x^T chunks stream on the SP ring in parallel.
            # W1 moves in 2-k-tile quarter panels so x^T chunks interleave into
            # the shared DMA engines instead of queueing behind 0.5MB blocks.
            KQ = 2
            NQ = KT // KQ

            def w1q_dma(tile_, pan, q):
                nc.scalar.dma_start(
                    out=tile_[:],
                    in_=w1_in[q * KQ * P:(q + 1) * KQ * P,
                              pan * MPP * P:(pan + 1) * MPP * P].rearrange(
                        "(k p) m -> p k m", p=P))

            w1_first = w1pool.tile([P, KQ, MPP * P], mybir.dt.bfloat16, tag="w1q0")
            w1q_dma(w1_first, 0, 0)

            # x^T in SBUF as separate 2-k-tile chunks, interleaved with the
            # panel-0 W1 quarters on the ACT ring so their arrival order at the
            # shared DMA engines alternates with the W1 stream
            xt_chunks = []
            q0_tiles = [w1_first]
            for k0 in range(0, KT, 2):
                xc = big.tile([P, 2, TOK], mybir.dt.bfloat16, tag=f"xt{k0}")
                (nc.sync if k0 == 0 else nc.scalar).dma_start(
                    out=xc[:],
                    in_=x_in[k0 * P:(k0 + 2) * P, :].rearrange("(k p) t -> p k t", p=P))
                xt_chunks.append(xc)
                if k0 + 2 < KT:
                    q = (k0 + 2) // 2
                    t = w1pool.tile([P, KQ, MPP * P], mybir.dt.bfloat16, tag=f"w1q{q}")
                    w1q_dma(t, 0, q)
                    q0_tiles.append(t)

            def xt(k):
                return xt_chunks[k // 2][:, k % 2, :]
            w2s = small.tile([P, MT], mybir.dt.float32)
            nc.sync.dma_start(out=w2s[:], in_=w2_in[:])
            b1s = small.tile([P, MT], mybir.dt.float32)
            nc.sync.dma_start(out=b1s[:], in_=b1_in[:])

            # stage-2 runs on the DVE: acc[p,t] += h_m[p,t] * W2[128m+p],
            # then one ones-vector matmul reduces acc over partitions.
            acc = big.tile([P, TOK], mybir.dt.float32)
            ones = small.tile([P, 1], mybir.dt.float32)
            nc.vector.memset(ones[:], 1.0)

            for pan in range(NPAN):
                if pan == 0:
                    quarters = q0_tiles
                else:
                    quarters = []
                    for q in range(NQ):
                        t = w1pool.tile([P, KQ, MPP * P], mybir.dt.bfloat16,
                                        tag=f"w1q{q}")
                        w1q_dma(t, pan, q)
                        quarters.append(t)
                for mi in range(MPP):
                    m = pan * MPP + mi
                    ps = psum.tile([P, TOK], mybir.dt.float32, tag="ps")
                    for k in range(KT):
                        lhsT = quarters[k // KQ][:, k % KQ, mi * P:(mi + 1) * P]
                        xk = xt(k)
                        for n0, nl in NSPLITS:
                            nc.tensor.matmul(ps[:, n0:n0 + nl], lhsT, xk[:, n0:n0 + nl],
                                             start=(k == 0), stop=(k == KT - 1))
                    # h = relu(r + b1) -> bf16
                    hm = w1pool.tile([P, TOK], mybir.dt.bfloat16, tag="hm")
                    nc.scalar.activation(out=hm[:], in_=ps[:],
                                         func=mybir.ActivationFunctionType.Relu,
                                         bias=b1s[:, m:m + 1], scale=1.0)
                    for n0, nl in NSPLITS:
                        if m == 0:
                            nc.vector.tensor_scalar_mul(out=acc[:, n0:n0 + nl],
                                                        in0=hm[:, n0:n0 + nl],
                                                        scalar1=w2s[:, 0:1])
                        else:
                            nc.vector.scalar_tensor_tensor(
                                out=acc[:, n0:n0 + nl], in0=hm[:, n0:n0 + nl],
                                scalar=w2s[:, m:m + 1],
                                in1=acc[:, n0:n0 + nl], op0=mybir.AluOpType.mult,
                                op1=mybir.AluOpType.add)

            # stage 2 partition-reduction: logits = ones^T @ acc, chunk-wise so
            # the first chunk's reduction overlaps the last chunk's accumulate
            zps = psum2.tile([P, TOK], mybir.dt.float32, tag="zps")
            zsb = small.tile([1, TOK], mybir.dt.float32)
            for n0, nl in NSPLITS:
                nc.tensor.matmul(zps[0:1, n0:n0 + nl], ones[:], acc[:, n0:n0 + nl],
                                 start=True, stop=True)
                nc.vector.tensor_copy(out=zsb[:, n0:n0 + nl], in_=zps[0:1, n0:n0 + nl])
                nc.sync.dma_start(out=z_out[:, n0:n0 + nl], in_=zsb[:, n0:n0 + nl])
    nc.compile()
    return nc


def _build_pool_nc(pattern):
    """Kernel B: per core, pooled[s, :] = mean of hidden-half over tokens with seg id s.

    pattern: tuple of (m, (k0, k1, ...)) — for each computed segment tile m, the
    token tiles whose segment-id range intersects it (a diagonal band since
    segment ids are monotone in token position).  Same program runs on all 8
    cores; the pattern is the union over the 4 samples.
    """
    nc = bacc.Bacc("TRN2", target_bir_lowering=False)
    mt = max(m for m, _ in pattern) + 1
    SEG = mt * P
    xh_in = nc.declare_dram_parameter("xh", [S, DH], mybir.dt.bfloat16, isOutput=False)
    xl_in = nc.declare_dram_parameter("xl", [S, DH], mybir.dt.bfloat16, isOutput=False)
    hh1_in = nc.declare_dram_parameter("hh1", [P, S // P], mybir.dt.float32, isOutput=False)
    invc_in = nc.declare_dram_parameter("invc", [P, mt], mybir.dt.float32, isOutput=False)
    iota_in = nc.declare_dram_parameter("iota", [P, SEG], mybir.dt.float32, isOutput=False)
    out = nc.declare_dram_parameter("pooled", [SEG, DH], mybir.dt.float32, isOutput=True)

    KT = S // P  # 16 token tiles
    # per token-tile k: contiguous run of segment tiles it feeds
    k_runs = {}
    for m, ks in pattern:
        for k in ks:
            lo, hi = k_runs.get(k, (m, m))
            k_runs[k] = (min(lo, m), max(hi, m))

    with tile.TileContext(nc) as tc:
        with (
            tc.tile_pool(name="big", bufs=1) as big,
            tc.tile_pool(name="ps", bufs=6, space="PSUM") as psum,
            tc.tile_pool(name="pw", bufs=1, space="PSUM") as psumw,
            tc.tile_pool(name="outp", bufs=4) as outp,
            tc.tile_pool(name="small", bufs=1) as small,
        ):
            warm = small.tile([P, 64], mybir.dt.bfloat16)
            nc.vector.memset(warm[:], 0.0)
            wps = psumw.tile([P, 64], mybir.dt.float32, tag="warm")
            for _ in range(40):
                nc.tensor.matmul(wps[0:64, :], warm[:], warm[:], start=True, stop=True)

            iota_rep = small.tile([P, SEG], mybir.dt.float32)
            nc.scalar.dma_start(out=iota_rep[:], in_=iota_in[:])
            hh1s = small.tile([P, KT], mybir.dt.float32)
            nc.scalar.dma_start(out=hh1s[:], in_=hh1_in[:])

            xh_chunks, xl_chunks = [], []
            for k0 in range(0, KT, 2):
                xhc = big.tile([P, 2, DH], mybir.dt.bfloat16, tag=f"xh{k0}")
                nc.sync.dma_start(
                    out=xhc[:],
                    in_=xh_in[k0 * P:(k0 + 2) * P, :].rearrange("(k p) d -> p k d", p=P))
                xlc = big.tile([P, 2, DH], mybir.dt.bfloat16, tag=f"xl{k0}")
                nc.sync.dma_start(
                    out=xlc[:],
                    in_=xl_in[k0 * P:(k0 + 2) * P, :].rearrange("(k p) d -> p k d", p=P))
                xh_chunks.append(xhc)
                xl_chunks.append(xlc)

            def xh(k):
                return xh_chunks[k // 2][:, k % 2, :]

            def xl(k):
                return xl_chunks[k // 2][:, k % 2, :]

            invcs = small.tile([P, mt], mybir.dt.float32)
            nc.scalar.dma_start(out=invcs[:], in_=invc_in[:])

            # one-hot band tiles: per k, one DVE op covering its segment-tile run
            maxrun = max(hi - lo + 1 for lo, hi in k_runs.values())
            ohk = big.tile([P, KT, maxrun * P], mybir.dt.bfloat16)
            oh = {}
            for k, (mlo, mhi) in sorted(k_runs.items()):
                nrun = mhi - mlo + 1
                nc.vector.tensor_scalar(out=ohk[:, k, :nrun * P],
                                        in0=iota_rep[:, mlo * P:(mhi + 1) * P],
                                        scalar1=hh1s[:, k:k + 1], scalar2=None,
                                        op0=mybir.AluOpType.is_equal)
                for m in range(mlo, mhi + 1):
                    oh[(m, k)] = ohk[:, k, (m - mlo) * P:(m - mlo + 1) * P]

            for m, ks in pattern:
                ps = psum.tile([P, DH], mybir.dt.float32, tag="ps")
                for j, k in enumerate(ks):
                    lhsT = oh[(m, k)][:]
                    nc.tensor.matmul(ps[:], lhsT, xh(k),
                                     start=(j == 0), stop=False)
                    nc.tensor.matmul(ps[:], lhsT, xl(k),
                                     start=False, stop=(j == len(ks) - 1))
                po = outp.tile([P, DH], mybir.dt.float32, tag="po")
                nc.scalar.activation(out=po[:], in_=ps[:],
                                     func=mybir.ActivationFunctionType.Copy,
                                     scale=invcs[:, m:m + 1])
                nc.sync.dma_start(out=out[m * P:(m + 1) * P, :], in_=po[:])
    nc.compile()
    return nc


TOK_FAST = 744            # tokens per core on the packed fast path
MAX_LEFTOVER = 256        # host-side exact-MLP budget for unpacked valid tokens


def kernel(hidden, attention_mask, noise_u, W1, b1, W2, b2):
    hidden = np.asarray(hidden, np.float32)
    attention_mask = np.asarray(attention_mask, np.float32)
    noise_u = np.asarray(noise_u, np.float32)
    W1 = np.asarray(W1, np.float32)
    b1 = np.asarray(b1, np.float32)
    W2 = np.asarray(W2, np.float32)
    b2 = np.float32(np.asarray(b2))

    # ---- host preprocessing -------------------------------------------------
    xflat = hidden.reshape(BS * S, D)
    xbf = xflat.astype(BF16)
    xlo = (xflat - xbf.astype(np.float32)).astype(BF16)
    w1bf = W1.astype(BF16)
    w2t = W2.reshape(H // P, P).T.copy()                    # [128, 32] f32
    b1t = b1.reshape(H // P, P).T.copy()                    # [128, 32] f32

    # ---- kernel A: logits ---------------------------------------------------
    valid_flat = (attention_mask.reshape(-1) > 0.0)
    vidx = np.nonzero(valid_flat)[0]
    V = vidx.size
    fast = V - N_CORES * TOK_FAST <= MAX_LEFTOVER
    tok_a = TOK_FAST if fast else TOK
    key_a = ("mlp", tok_a)
    if key_a not in _cache:
        _cache[key_a] = _Runner(_build_mlp_nc(tok_a), N_CORES)
    run_a = _cache[key_a]

    nbatch = N_CORES * tok_a
    if fast:
        dev_idx = vidx[:nbatch]
        if dev_idx.size < nbatch:  # pad with token 0
            dev_idx = np.concatenate(
                [dev_idx, np.zeros(nbatch - dev_idx.size, np.int64)])
        host_idx = vidx[nbatch:]
        xa = xbf[dev_idx]
    else:
        dev_idx = np.arange(nbatch)
        host_idx = np.zeros(0, np.int64)
        xa = xbf
    in_maps = [
        {
            "xbfT": np.ascontiguousarray(xa[c * tok_a:(c + 1) * tok_a].T),
            "w1": w1bf,
            "w2": w2t,
            "b1": b1t,
        }
        for c in range(N_CORES)
    ]
    res_a = run_a(in_maps)
    zdev = np.concatenate([res_a[c]["logits"][0] for c in range(N_CORES)])

    n_real = min(nbatch, V) if fast else nbatch
    logits = np.zeros(BS * S, np.float64)
    logits[dev_idx[:n_real]] = zdev.astype(np.float64)[:n_real]
    W1_64 = None
    if host_idx.size:
        W1_64 = W1.astype(np.float64)
        xr = hidden.reshape(-1, D)[host_idx].astype(np.float64)
        hr = np.maximum(xr @ W1_64 + b1.astype(np.float64), 0.0)
        logits[host_idx] = hr @ W2.astype(np.float64)
    logits = logits.reshape(BS, S)

    # ---- host: exact decisions ---------------------------------------------
    noise64 = noise_u.astype(np.float64)
    logistic = np.log(noise64) - np.log1p(-noise64)
    z = logits + float(b2) + logistic
    # recompute near-threshold tokens exactly (fp64).  The device bf16 logit
    # error scales with the logit magnitude; 0.03*std(z) keeps a ~5x margin
    # over the measured max error for unit-scale inputs.
    zvalid = z[attention_mask > 0.0]
    delta = max(FIXUP_DELTA, 0.03 * float(np.std(zvalid)) if zvalid.size else 0.0)
    risky = (np.abs(z) < delta) & (attention_mask > 0.0)
    if risky.any():
        rb, rs = np.nonzero(risky)
        if W1_64 is None:
            W1_64 = W1.astype(np.float64)
        xr = hidden[rb, rs].astype(np.float64)              # [n, D]
        hr = np.maximum(xr @ W1_64 + b1.astype(np.float64), 0.0)
        zr = hr @ W2.astype(np.float64) + float(b2) + logistic[rb, rs]
        z[rb, rs] = zr

    hard = ((z > 0.0) & (attention_mask > 0.0)).astype(np.int64)
    # forced boundary on last real token of each row (only when row has padding)
    lens = (attention_mask > 0.0).sum(1).astype(np.int64)
    for bi in range(BS):
        if 0 < lens[bi] < S:
            hard[bi, lens[bi] - 1] = 1
    hh1 = np.cumsum(hard, axis=1) - hard                    # segment id per token
    nseg = hh1[:, -1] + 1                                   # segments incl. trailing pad segment
    counts = np.stack([np.bincount(hh1[bi], minlength=S) for bi in range(BS)])

    num_b = np.float32(hard.sum())
    total = np.float32(attention_mask.sum())
    lg = math.lgamma
    log_prob = (lg(float(total) + 1.0) - lg(float(num_b) + 1.0)
                - lg(float(total) - float(num_b) + 1.0)
                + float(num_b) * math.log(PRIOR)
                + (float(total) - float(num_b)) * math.log(1.0 - PRIOR))
    loss = np.float32(-log_prob / float(total))
    counts_rows = hard.sum(1).astype(np.float32)
    short_mask = (np.arange(S, dtype=np.float32)[None, :] < counts_rows[:, None]).astype(np.float32)

    # ---- kernel B: segment mean-pool ---------------------------------------
    mt = (int(nseg.max()) + P - 1) // P
    SEG = mt * P
    # band pattern: union over samples of (segment-tile m, token-tile k) overlaps
    KT = S // P
    pairs = set()
    for bi in range(BS):
        lo = hh1[bi].reshape(KT, P).min(1) // P
        hi = hh1[bi].reshape(KT, P).max(1) // P
        for k in range(KT):
            for m in range(int(lo[k]), int(hi[k]) + 1):
                pairs.add((m, k))
    pattern = tuple(
        (m, tuple(sorted(k for mm, k in pairs if mm == m)))
        for m in sorted({mm for mm, _ in pairs})
    )
    key = ("pool", pattern)
    if key not in _cache:
        _cache[key] = _Runner(_build_pool_nc(pattern), N_CORES)
    run_b = _cache[key]

    invc = (np.float32(1.0) /
            (counts[:, :SEG].astype(np.float32) + np.float32(EPS)))  # [BS, SEG]
    hh1f = hh1.astype(np.float32)
    xbf3 = xbf.reshape(BS, S, D)
    xlo3 = xlo.reshape(BS, S, D)
    iota = np.ascontiguousarray(
        np.broadcast_to(np.arange(SEG, dtype=np.float32), (P, SEG)))

    in_maps_b = []
    for c in range(N_CORES):
        bi, half = c // 2, c % 2
        d0 = half * DH
        in_maps_b.append({
            "xh": np.ascontiguousarray(xbf3[bi, :, d0:d0 + DH]),
            "xl": np.ascontiguousarray(xlo3[bi, :, d0:d0 + DH]),
            "hh1": np.ascontiguousarray(hh1f[bi].reshape(KT, P).T),
            "invc": np.ascontiguousarray(invc[bi].reshape(mt, P).T),
            "iota": iota,
        })
    res_b = run_b(in_maps_b)

    pooled = np.zeros((BS, S, D), np.float32)
    for c in range(N_CORES):
        bi, half = c // 2, c % 2
        d0 = half * DH
        pooled[bi, :SEG, d0:d0 + DH] = res_b[c]["pooled"]

    return pooled, loss, num_b, total, short_mask
